# revision 6
# baseline (speedup 1.0000x reference)
"""Trainium2 Bass kernel for nn_ContinuousGenHyperConnections.

Sharding: data-parallel over the batch dim B=8192 across 8 NeuronCores
(1024 rows each). All weights replicated; no collectives.

Per-core dataflow (B_loc=1024 -> 8 b-tiles of 128 rows, 2 blocks of 4 tiles):
  P1 per tile : DMA x fp32 chunks, cast->bf16 (gpsimd) into the block-resident
                x_bf tile, squares+row-sum (DVE STT accum_out), store bf16
                copy to DRAM scratch (feeds the DMA-transpose loads).
  P2 per tile : s = rsqrt(mean(x^2)+eps)                     [128,1] f32
  P3 per block: xT [128d,512b] bf16 via DMA-transpose from scratch, 64-chunk
                accumulated matmul -> proj.T [42,512]  (W_cat row order:
                conv16|diss16|dtc|dtd|read4|write4), PE-transpose, scale by s.
  P4 per block: per-row 4x4 generator math batched over 4 tiles: skew, K=RR^T,
                dt sigmoids, A, expm (order-8 Taylor + 4 squarings), rw/ww,
                c = E^T rw.  fp32, strided/broadcast APs on DVE.
  P5 per tile : branch = sum_j c_j x_j (bf16), 16 PE-transposes -> branchT.
  P6 per block: y.T = W_mod @ branch.T, W_mod streamed per m-chunk (bf16,
                256 matmuls N=512), y.T bf16 -> DRAM.
  P7 per tile : y via DMA-transpose; out_n = sum_j E_nj x_j + ww_n y -> HBM.
"""

import os
import sys

sys.path.insert(0, "/opt/trn_rl_repo")

import numpy as np
import ml_dtypes

BF16 = ml_dtypes.bfloat16

DT_MIN, DT_MAX = 1e-3, 1.0
EPS = 1e-6
NS = 4  # streams
EMB = 2048
IN_DIM = 8192
N_CORES = 8
NPROJ = 42  # 16 conv + 16 diss + 1 dtc + 1 dtd + 4 read + 4 write


def _build(B_loc, scal, num_devices=N_CORES):
    import concourse.bacc as bacc
    import concourse.mybir as mybir
    import concourse.tile as tile
    from concourse.masks import make_identity
    from contextlib import ExitStack

    dt = mybir.dt
    Alu = mybir.AluOpType
    Act = mybir.ActivationFunctionType
    Axis = mybir.AxisListType

    NT = B_loc // 128
    TPB = min(4, NT)          # tiles per block
    NBLK = NT // TPB
    NCH = IN_DIM // 128       # 64 contraction chunks
    NB = TPB * 128            # rows per block

    # expm 2^-4 prescale folded into dt: dt_eff = (DT_MIN + range*sig)/16
    R_SIG = (DT_MAX - DT_MIN) / 16.0
    C_SIG = DT_MIN / 16.0

    nc = bacc.Bacc("TRN2", target_bir_lowering=False, debug=False,
                   num_devices=num_devices)

    x_ext = nc.declare_dram_parameter("x", [B_loc, IN_DIM], dt.float32,
                                      isOutput=False)
    wcatT_ext = nc.declare_dram_parameter("wcatT", [128, NCH, NPROJ],
                                          dt.bfloat16, isOutput=False)
    wmodT_ext = nc.declare_dram_parameter("wmodT", [16, 128, 16, 128],
                                          dt.bfloat16, isOutput=False)
    cpack_ext = nc.declare_dram_parameter("cpack", [58], dt.float32,
                                          isOutput=False)
    out_ext = nc.declare_dram_parameter("out", [B_loc, NS, EMB], dt.float32,
                                        isOutput=True)

    with tile.TileContext(nc) as tc, ExitStack() as ctx:
        const_pool = ctx.enter_context(tc.tile_pool(name="const", bufs=1))
        wm_pool = ctx.enter_context(tc.tile_pool(name="wm", bufs=2))
        dram_pool = ctx.enter_context(
            tc.tile_pool(name="dram", bufs=1, space="DRAM"))
        p1_pool = ctx.enter_context(tc.tile_pool(name="p1", bufs=2))
        xbb_pool = ctx.enter_context(tc.tile_pool(name="xbb", bufs=1))
        xt_pool = ctx.enter_context(tc.tile_pool(name="xt", bufs=3))
        small_pool = ctx.enter_context(tc.tile_pool(name="small", bufs=2))
        sm1_pool = ctx.enter_context(tc.tile_pool(name="sm1", bufs=1))
        str_pool = ctx.enter_context(tc.tile_pool(name="stream", bufs=2))
        brt_pool = ctx.enter_context(tc.tile_pool(name="brt", bufs=1))
        out_pool = ctx.enter_context(tc.tile_pool(name="outp", bufs=2))
        ps_proj = ctx.enter_context(
            tc.tile_pool(name="ps_proj", bufs=1, space="PSUM"))
        ps_tr = ctx.enter_context(
            tc.tile_pool(name="ps_tr", bufs=2, space="PSUM"))
        ps_y = ctx.enter_context(
            tc.tile_pool(name="ps_y", bufs=2, space="PSUM"))

        # ---- constants ----
        wcatT = const_pool.tile([128, NCH, NPROJ], dt.bfloat16)
        nc.sync.dma_start(wcatT[:], wcatT_ext[:])
        cpk = const_pool.tile([128, 58], dt.float32)
        nc.sync.dma_start(cpk[:], cpack_ext[:].partition_broadcast(128))
        ident_bf = const_pool.tile([128, 128], dt.bfloat16)
        make_identity(nc, ident_bf[:])
        ident_f32 = const_pool.tile([128, 128], dt.float32)
        make_identity(nc, ident_f32[:])

        skew_c = cpk[:, 0:16]     # (conservA+bconv) - transpose, flattened
        diss_c = cpk[:, 16:32]    # dissA + bdiss, flattened
        eye16 = cpk[:, 32:48]     # flattened I4
        readin_c = cpk[:, 48:52]
        writeout_c = cpk[:, 52:56]

        xbf_dram = dram_pool.tile([B_loc, IN_DIM], dt.bfloat16)
        y_dram = dram_pool.tile([EMB, B_loc], dt.bfloat16)

        s_all = sm1_pool.tile([128, NT], dt.float32)
        proj_all = sm1_pool.tile([128, NT, NPROJ], dt.float32)
        E_all = sm1_pool.tile([128, NT, 16], dt.float32)
        c_all = sm1_pool.tile([128, NT, NS], dt.float32)
        ww_all = sm1_pool.tile([128, NT, NS], dt.float32)

        def bcast(ap2d, shape):
            return ap2d.unsqueeze(1).broadcast_to(shape)

        for g in range(NBLK):
            rows = slice(g * NB, (g + 1) * NB)
            x_bf = xbb_pool.tile([128, TPB, IN_DIM], dt.bfloat16, tag="x_bf")

            # ============ P1/P2: load, cast, sum of squares ============
            for i in range(TPB):
                t = g * TPB + i
                ss = small_pool.tile([128, 4], dt.float32, tag="ss")
                for q in range(4):
                    xf = p1_pool.tile([128, EMB], dt.float32, tag="xf")
                    nc.sync.dma_start(
                        xf[:], x_ext[t * 128:(t + 1) * 128,
                                     q * EMB:(q + 1) * EMB])
                    nc.gpsimd.tensor_copy(x_bf[:, i, q * EMB:(q + 1) * EMB],
                                          xf[:])
                    sqs = p1_pool.tile([128, EMB], dt.bfloat16, tag="sqs")
                    nc.vector.scalar_tensor_tensor(
                        out=sqs[:], in0=x_bf[:, i, q * EMB:(q + 1) * EMB],
                        scalar=1.0, in1=x_bf[:, i, q * EMB:(q + 1) * EMB],
                        op0=Alu.mult, op1=Alu.mult,
                        accum_out=ss[:, q:q + 1])
                nc.sync.dma_start(xbf_dram[t * 128:(t + 1) * 128, :],
                                  x_bf[:, i, :])
                s01 = small_pool.tile([128, 1], dt.float32, tag="s01")
                s23 = small_pool.tile([128, 1], dt.float32, tag="s23")
                nc.vector.tensor_add(s01[:], ss[:, 0:1], ss[:, 1:2])
                nc.vector.tensor_add(s23[:], ss[:, 2:3], ss[:, 3:4])
                nc.vector.tensor_add(s01[:], s01[:], s23[:])
                nc.vector.tensor_scalar(
                    out=s01[:], in0=s01[:], scalar1=1.0 / IN_DIM,
                    scalar2=EPS, op0=Alu.mult, op1=Alu.add)
                sqr = small_pool.tile([128, 1], dt.float32, tag="sqr")
                nc.scalar.activation(sqr[:], s01[:], Act.Sqrt)
                nc.vector.reciprocal(s_all[:, t:t + 1], sqr[:])

            # ============ P3: proj.T = Wcat @ xn.T ============
            proj_ps = ps_proj.tile([NPROJ, NB], dt.float32, tag="proj_ps")
            for c in range(NCH):
                xt = xt_pool.tile([128, NB], dt.bfloat16, tag="xt")
                nc.sync.dma_start(
                    xt[:], xbf_dram[rows, c * 128:(c + 1) * 128],
                    transpose=True)
                nc.tensor.matmul(proj_ps[:], wcatT[:, c, :], xt[:],
                                 start=(c == 0), stop=(c == NCH - 1))
            projT = small_pool.tile([NPROJ, NB], dt.float32, tag="projT")
            nc.vector.tensor_copy(projT[:], proj_ps[:])
            for i in range(TPB):
                t = g * TPB + i
                tr_ps = ps_tr.tile([128, NPROJ], dt.float32, tag="tr_ps")
                nc.tensor.transpose(
                    tr_ps[:], projT[:, i * 128:(i + 1) * 128],
                    ident_f32[:NPROJ, :NPROJ])
                nc.vector.tensor_scalar(
                    out=proj_all[:, t, :], in0=tr_ps[:],
                    scalar1=s_all[:, t:t + 1], scalar2=None, op0=Alu.mult)

            # ============ P4: per-row generator math (batched) ============
            pb = proj_all[:, g * TPB:(g + 1) * TPB, :]   # [128,TPB,42]

            smw = small_pool.tile([128, TPB, 16], dt.float32, tag="smw")
            nc.vector.tensor_tensor(
                smw[:].rearrange("p t (i j) -> p t i j", j=NS),
                pb[:, :, 0:16].rearrange("p t (i j) -> p t i j", j=NS),
                pb[:, :, 0:16].rearrange("p t (j i) -> p t i j", i=NS),
                Alu.subtract)
            nc.vector.tensor_tensor(smw[:], smw[:],
                                    bcast(skew_c, [128, TPB, 16]), Alu.add)
            Rm = small_pool.tile([128, TPB, 16], dt.float32, tag="Rm")
            nc.vector.tensor_tensor(Rm[:], pb[:, :, 16:32],
                                    bcast(diss_c, [128, TPB, 16]), Alu.add)
            dtc = small_pool.tile([128, TPB, 1], dt.float32, tag="dtc")
            dtd = small_pool.tile([128, TPB, 1], dt.float32, tag="dtd")
            nc.scalar.activation(dtc[:], pb[:, :, 32:33], Act.Sigmoid,
                                 bias=cpk[:, 56:57])
            nc.scalar.activation(dtd[:], pb[:, :, 33:34], Act.Sigmoid,
                                 bias=cpk[:, 57:58])
            nc.vector.tensor_scalar(out=dtc[:], in0=dtc[:], scalar1=R_SIG,
                                    scalar2=C_SIG, op0=Alu.mult, op1=Alu.add)
            nc.vector.tensor_scalar(out=dtd[:], in0=dtd[:], scalar1=R_SIG,
                                    scalar2=C_SIG, op0=Alu.mult, op1=Alu.add)

            prod = small_pool.tile([128, TPB, 64], dt.float32, tag="prod")
            B4 = [128, NS, NS, NS]

            def mm_t(dst, lhs, rhs, rhs_pat, eng=nc.vector):
                # per-row 4x4 matmul: dst[(i,j)] = sum_k lhs[(i,k)]*rhs[pat]
                # (looped over the tile dim: ISA allows at most 3 free dims)
                for _i in range(TPB):
                    pv = prod[:, _i, :].rearrange("p (i j k) -> p i j k",
                                                  j=NS, k=NS)
                    eng.tensor_tensor(
                        pv,
                        lhs[:, _i, :].rearrange("p (i k) -> p i k", k=NS)
                            .unsqueeze(2).broadcast_to(B4),
                        rhs[:, _i, :].rearrange(rhs_pat, k=NS)
                            .unsqueeze(1).broadcast_to(B4),
                        Alu.mult)
                    eng.tensor_reduce(
                        dst[:, _i, :].rearrange("p (i j) -> p i j", j=NS),
                        pv, Axis.X, Alu.add)

            # K = R @ R^T
            Km = small_pool.tile([128, TPB, 16], dt.float32, tag="Km")
            mm_t(Km, Rm, Rm, "p (j k) -> p j k")
            # A = dtc*skew - dtd*K   (per-tile: dt scalars vary with t)
            Am = small_pool.tile([128, TPB, 16], dt.float32, tag="Am")
            for i in range(TPB):
                nc.vector.tensor_scalar(
                    out=Am[:, i, :], in0=Km[:, i, :],
                    scalar1=dtd[:, i, :], scalar2=None, op0=Alu.mult)
                nc.vector.scalar_tensor_tensor(
                    out=Am[:, i, :], in0=smw[:, i, :], scalar=dtc[:, i, :],
                    in1=Am[:, i, :], op0=Alu.mult, op1=Alu.subtract)
            # expm
            Em = small_pool.tile([128, TPB, 16], dt.float32, tag="Em")
            nc.vector.tensor_tensor(Em[:], Am[:],
                                    bcast(eye16, [128, TPB, 16]), Alu.add)
            term = small_pool.tile([128, TPB, 16], dt.float32, tag="term")
            term2 = small_pool.tile([128, TPB, 16], dt.float32, tag="term2")
            nc.vector.tensor_copy(term[:], Am[:])
            for k in range(2, 9):
                mm_t(term2, term, Am, "p (k j) -> p j k")
                nc.vector.tensor_scalar(out=term[:], in0=term2[:],
                                        scalar1=1.0 / k, scalar2=None,
                                        op0=Alu.mult)
                nc.vector.tensor_tensor(Em[:], Em[:], term[:], Alu.add)
            E2 = small_pool.tile([128, TPB, 16], dt.float32, tag="E2")
            cur, nxt = Em, E2
            for _ in range(4):
                mm_t(nxt, cur, cur, "p (k j) -> p j k")
                cur, nxt = nxt, cur
            nc.vector.tensor_copy(E_all[:, g * TPB:(g + 1) * TPB, :], cur[:])
            # rw / ww / c
            rw = small_pool.tile([128, TPB, NS], dt.float32, tag="rw")
            nc.vector.tensor_scalar(out=rw[:], in0=pb[:, :, 34:38],
                                    scalar1=scal["alpha_r"], scalar2=None,
                                    op0=Alu.mult)
            nc.vector.tensor_tensor(rw[:], rw[:],
                                    bcast(readin_c, [128, TPB, NS]), Alu.add)
            nc.scalar.activation(rw[:], rw[:], Act.Sigmoid)
            wws = ww_all[:, g * TPB:(g + 1) * TPB, :]
            nc.vector.tensor_scalar(out=wws, in0=pb[:, :, 38:42],
                                    scalar1=scal["alpha_w"], scalar2=None,
                                    op0=Alu.mult)
            nc.vector.tensor_tensor(wws, wws,
                                    bcast(writeout_c, [128, TPB, NS]),
                                    Alu.add)
            cprod = small_pool.tile([128, TPB, 16], dt.float32, tag="cprod")
            nc.vector.tensor_tensor(
                cprod[:].rearrange("p t (j n) -> p t j n", n=NS),
                cur[:].rearrange("p t (n j) -> p t j n", j=NS),
                rw[:].unsqueeze(2).broadcast_to([128, TPB, NS, NS]),
                Alu.mult)
            nc.vector.tensor_reduce(
                c_all[:, g * TPB:(g + 1) * TPB, :],
                cprod[:].rearrange("p t (j n) -> p t j n", n=NS),
                Axis.X, Alu.add)

            # ============ P5: branch + PE transposes ============
            brT = brt_pool.tile([128, 16, NB], dt.bfloat16, tag="brT")
            for i in range(TPB):
                t = g * TPB + i
                br = str_pool.tile([128, EMB], dt.bfloat16, tag="br")
                nc.scalar.activation(br[:], x_bf[:, i, 3 * EMB:4 * EMB],
                                     Act.Identity, scale=c_all[:, t, 3:4])
                for j in (2, 1, 0):
                    nc.vector.scalar_tensor_tensor(
                        out=br[:], in0=x_bf[:, i, j * EMB:(j + 1) * EMB],
                        scalar=c_all[:, t, j:j + 1], in1=br[:],
                        op0=Alu.mult, op1=Alu.add)
                for h in range(16):
                    br_ps = ps_tr.tile([128, 128], dt.bfloat16, tag="br_ps")
                    nc.tensor.transpose(br_ps[:],
                                        br[:, h * 128:(h + 1) * 128],
                                        ident_bf[:])
                    if h % 2 == 0:
                        nc.scalar.activation(
                            brT[:, h, i * 128:(i + 1) * 128], br_ps[:],
                            Act.Copy)
                    else:
                        nc.vector.tensor_copy(
                            brT[:, h, i * 128:(i + 1) * 128], br_ps[:])

            # ============ P6: y.T = Wmod @ branch.T ============
            for m in range(16):
                wm = wm_pool.tile([128, 16, 128], dt.bfloat16, tag="wm")
                nc.sync.dma_start(wm[:], wmodT_ext[m])
                y_ps = ps_y.tile([128, NB], dt.float32, tag="y_ps")
                for c in range(16):
                    nc.tensor.matmul(y_ps[:], wm[:, c, :], brT[:, c, :],
                                     start=(c == 0), stop=(c == 15))
                yT = small_pool.tile([128, NB], dt.bfloat16, tag="yT")
                if m % 2 == 0:
                    nc.scalar.activation(yT[:], y_ps[:], Act.Copy)
                else:
                    nc.vector.tensor_copy(yT[:], y_ps[:])
                nc.sync.dma_start(y_dram[m * 128:(m + 1) * 128, rows], yT[:])

            # ============ P7: outputs ============
            for i in range(TPB):
                t = g * TPB + i
                y_nb = str_pool.tile([128, EMB], dt.bfloat16, tag="y_nb")
                nc.sync.dma_start(y_nb[:],
                                  y_dram[:, t * 128:(t + 1) * 128],
                                  transpose=True)
                for n in range(NS):
                    u = str_pool.tile([128, EMB], dt.bfloat16, tag="u")
                    nc.scalar.activation(
                        u[:], x_bf[:, i, 0:EMB], Act.Identity,
                        scale=E_all[:, t, 4 * n:4 * n + 1])
                    eng = nc.vector
                    eng2 = nc.vector
                    for j in (1, 2):
                        eng.scalar_tensor_tensor(
                            out=u[:], in0=x_bf[:, i, j * EMB:(j + 1) * EMB],
                            scalar=E_all[:, t, 4 * n + j:4 * n + j + 1],
                            in1=u[:], op0=Alu.mult, op1=Alu.add)
                    eng2.scalar_tensor_tensor(
                        out=u[:], in0=x_bf[:, i, 3 * EMB:4 * EMB],
                        scalar=E_all[:, t, 4 * n + 3:4 * n + 4],
                        in1=u[:], op0=Alu.mult, op1=Alu.add)
                    ou = out_pool.tile([128, EMB], dt.float32, tag="ou")
                    eng.scalar_tensor_tensor(
                        out=ou[:], in0=y_nb[:],
                        scalar=ww_all[:, t, n:n + 1], in1=u[:],
                        op0=Alu.mult, op1=Alu.add)
                    nc.sync.dma_start(out_ext[t * 128:(t + 1) * 128, n, :],
                                      ou[:])

    nc.compile()
    return nc


def _prep_weights(inputs):
    W_conv = np.asarray(inputs["W_conv"], np.float32)
    W_diss = np.asarray(inputs["W_diss"], np.float32)
    W_dtc = np.asarray(inputs["W_dtc"], np.float32)
    W_dtd = np.asarray(inputs["W_dtd"], np.float32)
    W_read = np.asarray(inputs["W_read"], np.float32)
    W_write = np.asarray(inputs["W_write"], np.float32)
    W_mod = np.asarray(inputs["W_mod"], np.float32)

    Wcat = np.concatenate([W_conv, W_diss, W_dtc, W_dtd, W_read, W_write],
                          axis=0)
    assert Wcat.shape == (NPROJ, IN_DIM)
    wcatT = np.ascontiguousarray(
        Wcat.T.reshape(IN_DIM // 128, 128, NPROJ).transpose(1, 0, 2)
    ).astype(BF16)
    # [m, k-within-chunk, c, out-col]: element [m,p,c,q] = W_mod.T[c*128+p,
    # m*128+q] = W_mod[m*128+q, c*128+p]
    wmodT = np.ascontiguousarray(
        W_mod.T.reshape(16, 128, 16, 128).transpose(2, 1, 0, 3)
    ).astype(BF16)

    cM = np.asarray(inputs["conserv_A"], np.float32) + \
        np.asarray(inputs["b_conv"], np.float32).reshape(NS, NS)
    skew_const = (cM - cM.T).reshape(-1)
    dissC = (np.asarray(inputs["diss_A"], np.float32) +
             np.asarray(inputs["b_diss"], np.float32).reshape(NS, NS)
             ).reshape(-1)
    eye16 = np.eye(NS, dtype=np.float32).reshape(-1)
    readin = np.asarray(inputs["read_in"], np.float32).reshape(-1)
    writeout = np.asarray(inputs["write_out"], np.float32).reshape(-1)
    scal = dict(
        bias_c=float(np.asarray(inputs["log_dt_c"]).reshape(-1)[0]
                     + np.asarray(inputs["b_dtc"]).reshape(-1)[0]),
        bias_d=float(np.asarray(inputs["log_dt_d"]).reshape(-1)[0]
                     + np.asarray(inputs["b_dtd"]).reshape(-1)[0]),
        alpha_r=float(np.asarray(inputs["alpha_read_in"]).reshape(-1)[0]),
        alpha_w=float(np.asarray(inputs["alpha_write_out"]).reshape(-1)[0]),
    )
    cpack = np.concatenate([
        skew_const, dissC, eye16, readin, writeout,
        np.array([scal["bias_c"], scal["bias_d"]], np.float32)]
    ).astype(np.float32)
    assert cpack.shape == (58,)
    return wcatT, wmodT, cpack, scal


_NC_CACHE = {}


def kernel(**inputs):
    from concourse.bass_utils import run_bass_kernel_spmd

    x = np.asarray(inputs["x"], np.float32)
    B = x.shape[0]
    B_loc = B // N_CORES
    wcatT, wmodT, cpack, scal = _prep_weights(inputs)

    key = (B_loc, tuple(sorted(scal.items())))
    if key not in _NC_CACHE:
        _NC_CACHE[key] = _build(B_loc, scal)
    nc = _NC_CACHE[key]

    xf = x.reshape(B, IN_DIM)
    in_maps = []
    for i in range(N_CORES):
        in_maps.append({
            "x": np.ascontiguousarray(xf[i * B_loc:(i + 1) * B_loc]),
            "wcatT": wcatT,
            "wmodT": wmodT,
            "cpack": cpack,
        })

    trace = os.environ.get("KERNEL_TRACE", "0") == "1"
    res = run_bass_kernel_spmd(nc, in_maps, core_ids=list(range(N_CORES)),
                               trace=trace)
    if trace and res.exec_time_ns is not None:
        print(f"HW exec time: {res.exec_time_ns} ns")
        kernel.last_exec_time_ns = res.exec_time_ns
    out = np.concatenate([res.results[i]["out"] for i in range(N_CORES)],
                         axis=0)
    return out
